# revision 10
# baseline (speedup 1.0000x reference)
"""Trainium2 Bass kernel for nn_Distribution_74758200754679.

Computes, for x [65536, 8, 256] and a tiny MLP (256 -> 128 -> 1):
    h    = leaky_relu(x @ W1 + b1, 0.3)
    beta = sigmoid(h @ W2 + b2)            # [B, N]
    p    = stick_breaking(beta)            # [B, N+1]

Distribution: pure data parallel over 8 NeuronCores — x is sharded along
the batch axis, MLP params are replicated. Each core's shard is staged
host-side in transposed fp16 layout [128, 2, rows] (d_in split across
two K-halves on partitions) so each 4 MiB chunk DMA delivers complete
K for 8192 rows and the device loop is a chain of full-rate fp16
matmuls with half the HBM traffic of fp32 (the 2e-2 tolerance leaves
~50x headroom at fp16).

Per-core device program (32 MB of x per core, 64 pairs x 1024 rows):
  chunk DMA -> PE fp16 matmuls (K=256 accumulated in PSUM, [128,1024]
  2-bank tiles) -> ACT parametric-relu (one op per pair, bias=b1)
  -> PE rank-1 L2 matmuls packed 2-wide into PE column groups
  -> DVE [1,1024] PSUM->SBUF beta copies -> fan-out DMA to [block, row]
  -> tail per 64-block half: sigmoid + suffix-product stick-breaking.
The L2/copy stage is software-pipelined one pair behind L1 so the PE
never blocks on ACT.
"""

import os
import sys

# The device path runs through jax/PJRT on the neuron (axon) platform; a
# cpu-pinned JAX_PLATFORMS would hide the NeuronCores.
if os.environ.get("JAX_PLATFORMS") == "cpu":
    os.environ["JAX_PLATFORMS"] = ""

for _p in ("/opt/trn_rl_repo",):
    if _p not in sys.path:
        sys.path.insert(0, _p)

import numpy as np
from contextlib import ExitStack

import concourse.bacc as bacc
import concourse.mybir as mybir
from concourse import tile
from concourse import bass_utils

B, N, D_IN, D_H = 65536, 8, 256, 128
SLOPE = 0.3
CORES = 8
RC = B * N // CORES          # rows per core (65536)
BC = B // CORES              # batches per core (8192)
BLK = 512                    # rows per block
NBLK = RC // BLK             # 128
NPAIR = NBLK // 2            # 64
NG = BLK // N                # batch groups per partition in the tail (64)
GRP = 32                     # blocks per DMA chunk
DBLK = GRP * BLK             # 16384 rows per chunk (8 MiB fp16)
NCHUNK = NBLK // GRP         # 4
PPC = GRP // 2               # pairs per chunk (16)
HCB = 16                     # blocks per beta-staging half-chunk
NHC = NBLK // HCB            # 8 fan-out groups

f32 = mybir.dt.float32
f16 = mybir.dt.float16
AF = mybir.ActivationFunctionType
ALU = mybir.AluOpType

_NC_CACHE = []
_LAST_RESULTS = None


def _build():
    nc = bacc.Bacc(
        "TRN2", target_bir_lowering=False, debug=False, num_devices=CORES
    )
    xt_d = nc.dram_tensor("xt", [128, 2, RC], f16, kind="ExternalInput").ap()
    w1_d = nc.dram_tensor("w1", [128, 2, D_H], f16, kind="ExternalInput").ap()
    w2_d = nc.dram_tensor("w2", [D_H, 1], f16, kind="ExternalInput").ap()
    b1_d = nc.dram_tensor("b1v", [D_H, 1], f32, kind="ExternalInput").ap()
    st_d = nc.dram_tensor("st", [128, 1], f32, kind="ExternalInput").ap()
    nst_d = nc.dram_tensor("nst", [128, 1], f32, kind="ExternalInput").ap()
    p_d = nc.dram_tensor("p", [BC, N + 1], f32, kind="ExternalOutput").ap()

    with tile.TileContext(nc) as tc, ExitStack() as ctx:
        const = ctx.enter_context(tc.tile_pool(name="const", bufs=1))
        xpool = ctx.enter_context(tc.tile_pool(name="xp", bufs=1))
        hpool = ctx.enter_context(tc.tile_pool(name="hp", bufs=1))
        bpool = ctx.enter_context(tc.tile_pool(name="bp", bufs=1))
        tpool = ctx.enter_context(tc.tile_pool(name="tp", bufs=1))
        psh = ctx.enter_context(tc.tile_pool(name="psh", bufs=1, space="PSUM"))
        psb = ctx.enter_context(tc.tile_pool(name="psb", bufs=1, space="PSUM"))

        def T(pool, shape, dt_, nm, bufs=1):
            tag = nm.split("_")[0]
            return pool.tile(shape, dt_, name=nm, tag=tag, bufs=bufs)

        # x chunk tiles: both K-halves, one DMA per chunk (first chunk in
        # two halves so the pipeline fills ~5us sooner)
        xtiles = [None] * NCHUNK

        def load_chunk(c):
            x2 = T(xpool, [128, 2, DBLK], f16, f"x2_{c}", bufs=2)
            src = xt_d[:, :, c * DBLK:(c + 1) * DBLK]
            if c == 0:
                # pieces so the first pair starts ~3us after issue and the
                # stream stays just ahead of the rate-matched consumer
                bounds = (0, 1024, 4096, 8192, DBLK)
            elif c == 1:
                bounds = (0, DBLK // 2, DBLK)
            else:
                bounds = (0, DBLK)
            for lo, hi in zip(bounds[:-1], bounds[1:]):
                nc.sync.dma_start(x2[:, :, lo:hi], src[:, :, lo:hi])
            xtiles[c] = x2

        w1_sb = T(const, [128, 2, D_H], f16, "w1sb")
        nc.sync.dma_start(w1_sb[:], w1_d[:])
        w2_sb = T(const, [D_H, 1], f16, "w2sb")
        nc.sync.dma_start(w2_sb[:], w2_d[:])
        b1_sb = T(const, [D_H, 1], f32, "b1sb")
        nc.sync.dma_start(b1_sb[:], b1_d[:])
        st_sb = T(const, [128, 1], f32, "stsb")
        nc.sync.dma_start(st_sb[:], st_d[:])
        nst_sb = T(const, [128, 1], f32, "nstsb")
        nc.sync.dma_start(nst_sb[:], nst_d[:])

        load_chunk(0)

        # warm the ACT function tables (prelu+sigmoid set) while the first
        # x chunk is still in flight; also primes the DVE/ACT pipelines
        warm = T(const, [128, 1], f32, "warm")
        nc.vector.memset(warm[:], 0.25)
        nc.scalar.activation(warm[:], warm[:], AF.Prelu, bias=0.0, scale=1.0, alpha=SLOPE)
        nc.scalar.activation(warm[:], warm[:], AF.Sigmoid, bias=0.0, scale=1.0)

        # beta accumulator: partition = block index, free = row-in-block
        bt = T(bpool, [128, BLK], f16, "bt")
        # per-chunk beta staging rows on partition 0 (fp16: tail reads f16)
        bs_tiles = {}
        hh_tiles = {}
        pb_tiles = {}

        def emit_l1(q):
            """Pair q: 4 L1 matmuls into one [128,1024] PSUM tile + prelu."""
            c = q // PPC
            if q % PPC == 0:
                if c + 1 < NCHUNK and xtiles[c + 1] is None:
                    load_chunk(c + 1)
            x2 = xtiles[c]
            pl = q % PPC          # pair within chunk
            ca = slice((2 * pl) * BLK, (2 * pl + 1) * BLK)
            cb = slice((2 * pl + 1) * BLK, (2 * pl + 2) * BLK)
            ph2 = T(psh, [128, 2 * BLK], f32, f"ph2_{q}", bufs=2)
            nc.tensor.matmul(ph2[:, 0:BLK], w1_sb[:, 0, :], x2[:, 0, ca], start=True, stop=False)
            nc.tensor.matmul(ph2[:, BLK:2 * BLK], w1_sb[:, 0, :], x2[:, 0, cb], start=True, stop=False)
            nc.tensor.matmul(ph2[:, 0:BLK], w1_sb[:, 1, :], x2[:, 1, ca], start=False, stop=True)
            nc.tensor.matmul(ph2[:, BLK:2 * BLK], w1_sb[:, 1, :], x2[:, 1, cb], start=False, stop=True)
            hh2 = T(hpool, [128, 2 * BLK], f16, f"hh2_{q}", bufs=4)
            nc.scalar.activation(
                hh2[:], ph2[:], AF.Prelu, bias=b1_sb[:], scale=1.0, alpha=SLOPE
            )
            hh_tiles[q] = hh2

        def emit_l2(q, act_cast=False):
            """Pair q: rank-1 matmuls packed into PE col groups 0/32, plus
            beta-row drain copies + fan-out per 16-block half-chunk."""
            hc = q // 8           # half-chunk index (8 pairs each)
            hh2 = hh_tiles.pop(q)
            if q % 2 == 0:
                pb_tiles[q // 2] = T(psb, [128, 2 * BLK], f32, f"pb2_{q // 2}", bufs=2)
            pb2 = pb_tiles[q // 2]
            half = q % 2
            hs = slice(half * BLK, (half + 1) * BLK)
            nc.tensor.matmul(pb2[0:1, hs], w2_sb[:], hh2[:, 0:BLK],
                             start=True, stop=True, tile_position=(0, 0))
            nc.tensor.matmul(pb2[32:33, hs], w2_sb[:], hh2[:, BLK:2 * BLK],
                             start=True, stop=True, tile_position=(0, 32))
            if half == 1:
                if hc not in bs_tiles:
                    bs_tiles[hc] = T(bpool, [1, HCB * BLK], f16, f"bs_{hc}", bufs=2)
                bsv = bs_tiles[hc][:].rearrange("p (s r) -> p s r", r=BLK)
                pb2 = pb_tiles.pop(q // 2)
                # blocks in this pb2 tile: row 0 -> subs (2q-2, 2q) mod HCB,
                # row 32 -> +1 (free halves are consecutive pairs)
                s0 = (2 * (q - 1)) % HCB
                for j, row in ((0, 0), (1, 32)):
                    dst = bsv[0:1, s0 + j:s0 + j + 3:2, :]
                    srcap = pb2[row:row + 1, :].rearrange("p (a r) -> p a r", r=BLK)
                    if act_cast and j == 1:
                        nc.scalar.activation(dst, srcap, AF.Copy)
                    else:
                        nc.vector.tensor_copy(dst, srcap)
            if q % 8 == 7:
                # half-chunk complete: fan-out beta rows, one per partition
                # (SWDGE on the idle GpSimd engine; keeps ACT/sync queues clear)
                nc.gpsimd.dma_start(
                    bt[hc * HCB:(hc + 1) * HCB, :],
                    bs_tiles.pop(hc)[:].rearrange("p (j r) -> p j r", j=HCB),
                )

        sg = T(tpool, [128, BLK], f32, "sg")
        g = T(tpool, [128, BLK], f32, "g")
        s = T(tpool, [128, BLK], f32, "s")
        Pt = T(tpool, [128, NG * (N + 1)], f32, "P")

        def tail_a(h):
            """Sigmoids for block half h (bt partitions 64h..64h+63)."""
            P = slice(64 * h, 64 * (h + 1))
            nc.scalar.activation(sg[P, :], bt[P, :], AF.Sigmoid,
                                 bias=st_sb[P, :], scale=1.0)
            nc.scalar.activation(g[P, :], bt[P, :], AF.Sigmoid,
                                 bias=nst_sb[P, :], scale=-1.0)

        def tail_b(h):
            """Suffix products s[e] = prod_{k>=e} g[k] (log-tree; forward
            refs read ahead of writes on DVE)."""
            P = slice(64 * h, 64 * (h + 1))
            sv = s[:].rearrange("p (gr e) -> p gr e", e=N)
            gv = g[:].rearrange("p (gr e) -> p gr e", e=N)
            nc.vector.tensor_mul(sv[P, :, 0:N - 1], gv[P, :, 0:N - 1], gv[P, :, 1:N])
            nc.vector.tensor_copy(sv[P, :, N - 1:N], gv[P, :, N - 1:N])
            for k in (2, 4):
                nc.vector.tensor_mul(sv[P, :, 0:N - k], sv[P, :, 0:N - k], sv[P, :, k:N])

        def tail_c(h):
            """P assembly + output DMA for block half h."""
            P = slice(64 * h, 64 * (h + 1))
            sv = s[:].rearrange("p (gr e) -> p gr e", e=N)
            Pv = Pt[:].rearrange("p (gr e) -> p gr e", e=N + 1)
            sgv = sg[:].rearrange("p (gr e) -> p gr e", e=N)
            nc.vector.tensor_copy(Pv[P, :, 0:1], sv[P, :, 0:1])
            nc.vector.tensor_mul(Pv[P, :, 1:N], sgv[P, :, 0:N - 1], sv[P, :, 1:N])
            nc.vector.tensor_copy(Pv[P, :, N:N + 1], sgv[P, :, N - 1:N])
            nc.gpsimd.dma_start(
                p_d[64 * h * NG:(64 * h + 64) * NG, :]
                .rearrange("(blk gr) e -> blk (gr e)", gr=NG),
                Pt[P, :],
            )

        # L2/cast stage trails L1 by TWO pairs so the PE never waits on the
        # ACT prelu, even when the scheduler hoists L2 ahead of L1 at chunk
        # boundaries. Half-0 tail is staged well after its fan-outs land so
        # scheduler hoisting cannot stall the ACT queue.
        for q in range(NPAIR):
            emit_l1(q)
            if q >= 2:
                emit_l2(q - 2)
            if q == 44:
                tail_a(0)
            elif q == 46:
                tail_b(0)
            elif q == 48:
                tail_c(0)
        emit_l2(NPAIR - 2, act_cast=True)
        emit_l2(NPAIR - 1, act_cast=True)
        tail_a(1)
        tail_b(1)
        tail_c(1)

    nc.compile()
    return nc


def _get_nc():
    if not _NC_CACHE:
        _NC_CACHE.append(_build())
    return _NC_CACHE[0]


def kernel(**inputs):
    x = np.asarray(inputs["x"], dtype=np.float32)
    W1 = np.ascontiguousarray(np.asarray(inputs["W1"], dtype=np.float32))
    b1 = np.asarray(inputs["b1"], dtype=np.float32)
    W2 = np.ascontiguousarray(np.asarray(inputs["W2"], dtype=np.float32))
    b2 = np.asarray(inputs["b2"], dtype=np.float32)

    nc = _get_nc()

    xf = x.reshape(B * N, D_IN)
    st_val = np.float32(float(b2[0]))
    b1v = np.ascontiguousarray(b1.reshape(D_H, 1).astype(np.float32))
    stv = np.full((128, 1), st_val, np.float32)
    nstv = np.ascontiguousarray(-stv)
    # w1 pre-rearranged host-side: [256,128] -> [128 part, 2 khalf, 128 m]
    w1h = np.ascontiguousarray(
        W1.astype(np.float16).reshape(2, 128, D_H).transpose(1, 0, 2)
    )
    w2h = W2.astype(np.float16)

    in_maps = []
    for c in range(CORES):
        shard = xf[c * RC:(c + 1) * RC]
        # [rows, 256] -> [256, rows] -> [2, 128, rows] -> [128, 2, rows]
        xt = np.ascontiguousarray(
            shard.T.astype(np.float16).reshape(2, 128, RC).transpose(1, 0, 2)
        )
        in_maps.append({
            "xt": xt, "w1": w1h, "w2": w2h,
            "b1v": b1v, "st": stv, "nst": nstv,
        })

    res = bass_utils.run_bass_kernel_spmd(
        nc, in_maps, core_ids=list(range(CORES))
    )
    global _LAST_RESULTS
    _LAST_RESULTS = res
    p = np.concatenate(
        [res.results[c]["p"] for c in range(CORES)], axis=0
    ).astype(np.float32)
    return p
